# revision 46
# baseline (speedup 1.0000x reference)
"""Trainium2 Bass kernel for nn_MinimalSSM: selective-scan SSM block.

Reference computation (per batch b):
    proj  = x @ W_xproj + b_xproj                # [L, d+2n]
    delta = softplus(proj[:, :d])                # [L, d]
    Bm, Cm = proj[:, d:d+n], proj[:, d+n:]       # [L, n]
    A     = -exp(A_log)                          # [d, n]
    h_t   = exp(delta_t*A) * h_{t-1} + delta_t*Bm_t*x_t   (elementwise [d, n])
    y_t   = sum_n(h_t * Cm_t) + Dp * x_t
    out   = y @ W_out + b_out

Sharding (8 cores): batch (4) x d_model-half (2).  Each core computes the
full recurrence for its 512 channels of its batch, and a partial
out-projection (contraction over its d-half).  A second tiny kernel adds the
two partials per batch (sharded batch x out-column-half).

The time recurrence runs on VectorE's native tensor_tensor_scan
(state = a*state + b along the free dim, fp32 internal state).
exp(delta*A) is computed on ScalarE as activation(Exp, scale=A[:,n]) --
per-partition vector scale -- so the DVE only does the multiplies,
scan, and the n-reduction tree.

The Dp*x skip term is folded into the out-projection on the PE:
out = y @ W_out + x @ (diag(Dp) W_out) + b_out, with W2 = diag(Dp) W_out
precomputed on the host.  This removes the per-chunk DVE STT pass.

The scan a/b tiles are persistent per channel-block: the two chained-scan
dummy columns per n-segment (a = (0, 1), b = (carry, 0)) are written once
at start; per chunk ScalarE writes the new carry directly into b[:, :, 0:1]
(legal because the previous scan has already consumed the tile), removing
all per-chunk DVE copies.

Layout inside a core: partition dim = 128-channel block (4 blocks),
free dims = (n=16, t=Tc) per time-chunk, t innermost/contiguous.
"""

import numpy as np
import ml_dtypes

import concourse.bacc as bacc
import concourse.bass as bass
import concourse.tile as tile
from concourse import mybir
from concourse.bass_utils import run_bass_kernel_spmd
from contextlib import ExitStack

F32 = mybir.dt.float32
BF16 = mybir.dt.bfloat16
FP16 = mybir.dt.float16
AF = mybir.ActivationFunctionType
OP = mybir.AluOpType

B, T, D, N = 4, 2048, 1024, 16
DL = D // 2          # channels per core
NJB = DL // 128      # 4 local channel blocks
NKB = D // 128       # 8 contraction blocks for proj
NEB = D // 128       # 8 output-column blocks
PC = 544             # proj columns per core: 512 delta + 16 B + 16 C
TC = 256             # time chunk
NCH = T // TC
TCP = TC + 2         # scan chain length per n-segment (2 dummy cols)

_cache = {}


def _pin_act_tables():
    """Restrict bacc's activation-table choices to the one set containing
    every function we use (Exp, Ln, Identity, MemsetZero) so the compiler
    never inserts mid-kernel ACT_TABLE_LOAD switches."""
    import concourse.bacc as _bacc_mod
    from concourse.hw_specs import get_activation_tables as _orig

    def _only_nl_exp(arch):
        tabs = _orig(arch)
        # keep every entry (act_func_set_id is positional) but empty out the
        # alternatives so the chooser can only pick the one full set
        return {k: (v if k == "natural_log_exp_and_others" else set())
                for k, v in tabs.items()}

    _bacc_mod.get_activation_tables = _only_nl_exp


_pin_act_tables()


def _build_stage1(t_len=T, tc=TC):
    nch = t_len // tc
    tcp = tc + 2
    nc = bacc.Bacc("TRN2", target_bir_lowering=False, debug=False, num_devices=8)
    xt = nc.dram_tensor("xt", [D, t_len], BF16, kind="ExternalInput")
    xt0 = nc.dram_tensor("xt0", [128, NKB * tc], BF16, kind="ExternalInput")
    wx = nc.dram_tensor("wx", [D, PC], BF16, kind="ExternalInput")
    bx = nc.dram_tensor("bx", [128, 5], F32, kind="ExternalInput")
    alog = nc.dram_tensor("alog", [128, NJB * N], F32, kind="ExternalInput")
    wo = nc.dram_tensor("wo", [DL, D], BF16, kind="ExternalInput")
    wo2 = nc.dram_tensor("wo2", [DL, D], BF16, kind="ExternalInput")
    bo = nc.dram_tensor("bo", [128, NEB], F32, kind="ExternalInput")
    part = nc.dram_tensor("part", [D, t_len], FP16, kind="ExternalOutput")
    bc_dram = nc.dram_tensor("bc_scratch", [t_len // tc, 32, tc], BF16)

    with tile.TileContext(nc) as tc_ctx, ExitStack() as ctx:
        const = ctx.enter_context(tc_ctx.tile_pool(name="const", bufs=1))
        psum = ctx.enter_context(
            tc_ctx.tile_pool(name="psum", bufs=8, space="PSUM"))
        epool = ctx.enter_context(tc_ctx.tile_pool(name="etmp", bufs=2))
        dpool = ctx.enter_context(tc_ctx.tile_pool(name="delta", bufs=6))
        dxpool = ctx.enter_context(tc_ctx.tile_pool(name="dx", bufs=6))
        bcpool = ctx.enter_context(tc_ctx.tile_pool(name="bc", bufs=2))
        reppool = ctx.enter_context(tc_ctx.tile_pool(name="rep", bufs=2))
        hpool = ctx.enter_context(tc_ctx.tile_pool(name="hpool", bufs=2))
        ypool = ctx.enter_context(tc_ctx.tile_pool(name="y", bufs=8))
        popool = ctx.enter_context(tc_ctx.tile_pool(name="po", bufs=2))

        # --- weights / small constants on the Sync queue, interleaved with
        # the first chunk of x so the first proj matmul starts immediately;
        # bulk x / out-proj weights ride the Activation HWDGE queue so they
        # never delay the per-chunk bc/brep broadcast stream on Sync ---
        # Every dma_start costs ~0.65us of issue time on its HWDGE engine's
        # queue, so keep the Activation queue completely free of triggers
        # (it must start the softplus/exp stream immediately) and issue
        # everything from Sync.  Only chunk 0's inputs load up front; the
        # bulk x tail and out-proj weights are triggered from inside
        # chunk 0 (see the ci == 0 block below) so they never delay the
        # first proj matmul or the chunk-0 broadcast.
        bx_sb = const.tile([128, 5], F32, tag="bx")
        nc.sync.dma_start(bx_sb[:], bx[:])
        alog_sb = const.tile([128, NJB * N], F32, tag="alog")
        nc.sync.dma_start(alog_sb[:], alog[:])
        xt0s = const.tile([128, NKB, tc], BF16, tag="xt0s")
        nc.sync.dma_start(xt0s[:].rearrange("p k t -> p (k t)"), xt0[:])
        wx_sb = []
        xt_sb = []
        for kb in range(NKB):
            tt = const.tile([128, PC], BF16, tag=f"wx{kb}")
            nc.sync.dma_start(tt[:], wx[kb * 128:(kb + 1) * 128, :])
            wx_sb.append(tt)
            tt = const.tile([128, t_len], BF16, tag=f"xt{kb}")
            xt_sb.append(tt)
        wo_sb = []
        for kb in range(NJB):
            tt = const.tile([128, D], BF16, tag=f"wo{kb}")
            wo_sb.append(tt)
        wo2_sb = []
        for kb in range(NJB):
            tt = const.tile([128, D], BF16, tag=f"wo2{kb}")
            wo2_sb.append(tt)
        bo_sb = const.tile([128, NEB], F32, tag="bo")
        aexp_sb = const.tile([128, NJB * N], F32, tag="aexp")
        nc.scalar.activation(aexp_sb[:], alog_sb[:], AF.Exp)
        aneg_sb = const.tile([128, NJB * N], F32, tag="aneg")
        nc.vector.tensor_scalar_mul(aneg_sb[:], aexp_sb[:], -1.0)

        # Persistent scan operand tiles, one (a, b) pair per channel block.
        # Dummy columns written once: a = (0, 1) resets the running state to
        # the injected carry and passes it through; b = (carry, 0) carries
        # the previous chunk's final state (0 for the first chunk).
        at_t = []
        bt_t = []
        for jb in range(NJB):
            at = const.tile([128, N, tcp], BF16, tag=f"at{jb}")
            nc.vector.memset(at[:, :, 0:1], 0.0)
            nc.vector.memset(at[:, :, 1:2], 1.0)
            at_t.append(at)
            bt = const.tile([128, N, tcp], BF16, tag=f"bt{jb}")
            nc.vector.memset(bt[:, :, 0:2], 0.0)
            bt_t.append(bt)

        def emit_outproj(pybf, pt0):
            for eb in range(NEB):
                pso = psum.tile([128, tc], F32, tag="ps", name=f"pso{eb}")
                for kb in range(NJB):
                    for s in range(4):
                        nc.tensor.matmul(
                            pso[:],
                            wo_sb[kb][:, eb * 128:(eb + 1) * 128],
                            pybf[kb][:, s, :],
                            start=(kb == 0 and s == 0), stop=False)
                for kb in range(NJB):
                    nc.tensor.matmul(
                        pso[:],
                        wo2_sb[kb][:, eb * 128:(eb + 1) * 128],
                        xt_sb[kb][:, pt0:pt0 + tc],
                        start=False, stop=(kb == NJB - 1))
                pot = popool.tile([128, tc], FP16, tag="po")
                nc.scalar.activation(pot[:], pso[:], AF.Identity,
                                     bias=bo_sb[:, eb:eb + 1])
                nc.sync.dma_start(
                    part[eb * 128:(eb + 1) * 128, pt0:pt0 + tc], pot[:])

        pending_out = None
        for ci in range(nch):
            t0 = ci * tc
            xs = ((lambda kb: xt0s[:, kb, :]) if ci == 0
                  else (lambda kb: xt_sb[kb][:, t0:t0 + tc]))
            # B/C projection first: unblocks the broadcasts
            ps = psum.tile([32, tc], F32, tag="ps")
            for kb in range(NKB):
                nc.tensor.matmul(
                    ps[:],
                    wx_sb[kb][:, 512:544],
                    xs(kb),
                    start=(kb == 0), stop=(kb == NKB - 1))
            bct = bcpool.tile([32, tc], BF16, tag="bc")
            nc.scalar.activation(bct[:], ps[:], AF.Identity,
                                 bias=bx_sb[:32, 4:5])
            nc.sync.dma_start(bc_dram[ci], bct[:])
            brep = reppool.tile([128, N, tc], BF16, tag="brep")
            crep = reppool.tile([128, N, tc], BF16, tag="crep")
            nc.sync.dma_start(brep[:],
                              bc_dram[ci, 0:N, :].partition_broadcast(128))
            # chunk 0: crep rides the Scalar DGE so brep's descriptor
            # generation + transfer (the first b-multiply's gate) finishes
            # ~7us earlier on Sync; costs one 0.65us trigger before sp(jb0)
            cq = nc.scalar if ci == 0 else nc.sync
            cq.dma_start(crep[:],
                         bc_dram[ci, N:2 * N, :].partition_broadcast(128))
            if ci == 0:
                # bulk loads triggered from inside chunk 0: the Sync queue
                # has already issued everything chunk 0 needs, and these
                # all land long before their first consumers (chunk 1's
                # proj / the first deferred out-projection)
                for kb in range(NKB):
                    nc.sync.dma_start(xt_sb[kb][:],
                                      xt[kb * 128:(kb + 1) * 128, :])
                for kb in range(NJB):
                    nc.sync.dma_start(wo_sb[kb][:],
                                      wo[kb * 128:(kb + 1) * 128, :])
                for kb in range(NJB):
                    nc.sync.dma_start(wo2_sb[kb][:],
                                      wo2[kb * 128:(kb + 1) * 128, :])
                nc.sync.dma_start(bo_sb[:], bo[:])

            y_tiles = []
            carry_jobs = []
            for jb in range(NJB):
                # delta projection + softplus for this block only, so jb0's
                # exp stream starts three softpluses earlier on ScalarE
                ps = psum.tile([128, tc], F32, tag="ps")
                for kb in range(NKB):
                    nc.tensor.matmul(
                        ps[:],
                        wx_sb[kb][:, jb * 128:(jb + 1) * 128],
                        xs(kb),
                        start=(kb == 0), stop=(kb == NKB - 1))
                et_ = epool.tile([128, tc], F32, tag="etmp")
                nc.scalar.activation(et_[:], ps[:], AF.Exp,
                                     bias=bx_sb[:, jb:jb + 1])
                dt_ = dpool.tile([128, tc], BF16, tag="delta")
                nc.scalar.activation(dt_[:], et_[:], AF.Ln, bias=1.0)
                at = at_t[jb]
                bt = bt_t[jb]
                dxt = dxpool.tile([128, tc], BF16, tag="dx")
                nc.vector.tensor_mul(dxt[:], dt_[:], xs(jb))
                for n in range(N):
                    nc.scalar.activation(
                        at[:, n, 2:], dt_[:], AF.Exp,
                        scale=aneg_sb[:, jb * N + n:jb * N + n + 1])
                dx_b = dxt[:].unsqueeze(1).broadcast_to([128, N, tc])
                nc.vector.tensor_mul(bt[:, :, 2:], dx_b, brep[:])
                ht = hpool.tile([128, N, tcp], BF16, tag="h")
                nc.vector.tensor_tensor_scan(
                    ht[:].rearrange("p n t -> p (n t)"),
                    at[:].rearrange("p n t -> p (n t)"),
                    bt[:].rearrange("p n t -> p (n t)"),
                    0.0, op0=OP.mult, op1=OP.add)
                if ci < nch - 1:
                    carry_jobs.append((jb, ht))
                # write the PREVIOUS jb's carry in place on ACT: by now its
                # scan is long done, so this never stalls ScalarE's exp
                # stream, and the next chunk's scan sees the fresh carry
                if len(carry_jobs) >= 2:
                    pjb, pht = carry_jobs.pop(0)
                    nc.scalar.activation(bt_t[pjb][:, :, 0:1],
                                         pht[:, :, tcp - 1:tcp], AF.Identity)
                # h*C lands in bt's payload region (dead once the scan has
                # read it); two tree levels reduce 16 -> 4 partial sums
                # (ping-pong through ht's cols 0:tc, dead except the carry
                # col tcp-1).  The final 4 -> 1 reduction is folded into the
                # out-projection matmul: its PSUM output AP revisits the
                # same bank region with a stride-0 dim, so the PE
                # accumulates the 4 slices for free.
                nc.vector.tensor_mul(bt[:, :, 2:], ht[:, :, 2:], crep[:])
                nc.vector.tensor_add(ht[:, 0:8, 0:tc], bt[:, 0:8, 2:],
                                     bt[:, 8:16, 2:])
                y4 = ypool.tile([128, 4, tc], BF16, tag="y")
                nc.vector.tensor_add(y4[:], ht[:, 0:4, 0:tc],
                                     ht[:, 4:8, 0:tc])
                if ci == nch - 1:
                    # final chunk: finish the reduction on the DVE so the
                    # serial tail out-projection is only 8 passes per eb
                    nc.vector.tensor_add(ht[:, 0:2, 0:tc], y4[:, 0:2, :],
                                         y4[:, 2:4, :])
                    yt = dxpool.tile([128, tc], BF16, tag="dx")
                    nc.vector.tensor_add(yt[:], ht[:, 0, 0:tc],
                                         ht[:, 1, 0:tc])
                    y_tiles.append(yt)
                else:
                    y_tiles.append(y4)

            while carry_jobs:
                pjb, pht = carry_jobs.pop(0)
                nc.scalar.activation(bt_t[pjb][:, :, 0:1],
                                     pht[:, :, tcp - 1:tcp], AF.Identity)

            # previous chunk's tail: out-projection (PE) + evac (ACT).
            # Deferred one chunk so none of it sits on this chunk's critical
            # path for any engine.  The Dp*x skip connection rides along as
            # 4 extra accumulation passes against W2 = diag(Dp) W_out.
            if pending_out is not None:
                emit_outproj(*pending_out)
            pending_out = (y_tiles, t0)

        # final chunk's out-projection: all x-passes first (they depend
        # only on xt, so the PE runs them while the last scans are still in
        # flight), leaving only the y-passes and evacs on the serial tail
        pybf, pt0 = pending_out
        fpso = [psum.tile([128, tc], F32, tag="ps", name=f"fpso{eb}")
                for eb in range(NEB)]
        for eb in range(NEB):
            for kb in range(NJB):
                nc.tensor.matmul(
                    fpso[eb][:],
                    wo2_sb[kb][:, eb * 128:(eb + 1) * 128],
                    xt_sb[kb][:, pt0:pt0 + tc],
                    start=(kb == 0), stop=False)
        for eb in range(NEB):
            for kb in range(NJB):
                nc.tensor.matmul(
                    fpso[eb][:],
                    wo_sb[kb][:, eb * 128:(eb + 1) * 128],
                    pybf[kb][:],
                    start=False, stop=(kb == NJB - 1))
            pot = popool.tile([128, tc], FP16, tag="po")
            nc.scalar.activation(pot[:], fpso[eb][:], AF.Identity,
                                 bias=bo_sb[:, eb:eb + 1])
            nc.sync.dma_start(
                part[eb * 128:(eb + 1) * 128, pt0:pt0 + tc], pot[:])
    nc.compile()
    return nc


def _build_stage2(t_len=T):
    nc = bacc.Bacc("TRN2", target_bir_lowering=False, debug=False, num_devices=8)
    p0 = nc.dram_tensor("p0", [DL, t_len], FP16, kind="ExternalInput")
    p1 = nc.dram_tensor("p1", [DL, t_len], FP16, kind="ExternalInput")
    s = nc.dram_tensor("s", [128, DL // 128, t_len], FP16, kind="ExternalOutput")
    tcw = 2048
    nkb = DL // 128
    with tile.TileContext(nc) as tc_ctx, ExitStack() as ctx:
        pool = ctx.enter_context(tc_ctx.tile_pool(name="p", bufs=12))
        # All loads issued up front — p0 on the Sync HWDGE queue, p1 on the
        # Activation one — so the two input streams transfer concurrently
        # and no store (which waits on its add) ever blocks a later load.
        a_ts, b_ts = [], []
        for kb in range(nkb):
            a_t = pool.tile([128, tcw], FP16, tag="a")
            nc.sync.dma_start(a_t[:], p0[kb * 128:(kb + 1) * 128, :])
            a_ts.append(a_t)
            b_t = pool.tile([128, tcw], FP16, tag="b")
            nc.scalar.dma_start(b_t[:], p1[kb * 128:(kb + 1) * 128, :])
            b_ts.append(b_t)
        for kb in range(nkb):
            o_t = pool.tile([128, tcw], FP16, tag="o")
            nc.vector.tensor_add(o_t[:], a_ts[kb][:], b_ts[kb][:])
            nc.sync.dma_start(s[:, kb, :], o_t[:])
    nc.compile()
    return nc


def _stage1_inputs(x, A_log, Dp, W_xproj, b_xproj, W_out, b_out):
    bf = ml_dtypes.bfloat16
    W2 = Dp[:, None] * W_out
    in_maps = []
    for c in range(8):
        b, j = c % 4, c // 4
        lo, hi = j * DL, (j + 1) * DL
        order = np.concatenate(
            [np.arange(lo, hi), np.arange(0, lo), np.arange(hi, D)])
        cols = np.concatenate([np.arange(lo, hi), np.arange(D, D + 2 * N)])
        xt_full = np.ascontiguousarray(x[b].T[order]).astype(bf)
        xt0_l = np.ascontiguousarray(
            xt_full[:, 0:TC].reshape(NKB, 128, TC).transpose(1, 0, 2)
            .reshape(128, NKB * TC))
        wxc = np.ascontiguousarray(W_xproj[order][:, cols]).astype(bf)
        bx_pad = np.zeros(5 * 128, np.float32)
        bx_pad[:PC] = b_xproj[cols]
        bx_arr = np.ascontiguousarray(bx_pad.reshape(5, 128).T)
        alog_l = np.ascontiguousarray(
            A_log[lo:hi].reshape(NJB, 128, N).transpose(1, 0, 2).reshape(128, NJB * N))
        wo_l = np.ascontiguousarray(W_out[lo:hi]).astype(bf)
        wo2_l = np.ascontiguousarray(W2[lo:hi]).astype(bf)
        bo_src = b_out if j == 0 else np.zeros_like(b_out)
        bo_l = np.ascontiguousarray(bo_src.reshape(NEB, 128).T.astype(np.float32))
        in_maps.append({
            "xt": xt_full, "xt0": xt0_l, "wx": wxc, "bx": bx_arr,
            "alog": alog_l,
            "wo": wo_l, "wo2": wo2_l, "bo": bo_l,
        })
    return in_maps


def kernel(x, A_log, Dp, W_xproj, b_xproj, W_out, b_out, _trace=False):
    x = np.asarray(x, np.float32)
    A_log = np.asarray(A_log, np.float32)
    Dp = np.asarray(Dp, np.float32)
    W_xproj = np.asarray(W_xproj, np.float32)
    b_xproj = np.asarray(b_xproj, np.float32)
    W_out = np.asarray(W_out, np.float32)
    b_out = np.asarray(b_out, np.float32)

    if "s1" not in _cache:
        _cache["s1"] = _build_stage1()
    if "s2" not in _cache:
        _cache["s2"] = _build_stage2()

    in1 = _stage1_inputs(x, A_log, Dp, W_xproj, b_xproj, W_out, b_out)
    kw = dict(trace=True, trace_cores=list(range(8))) if _trace else {}
    res1 = run_bass_kernel_spmd(_cache["s1"], in1, core_ids=list(range(8)), **kw)
    parts = [res1.results[c]["part"] for c in range(8)]

    in2 = []
    for c in range(8):
        b, eh = c % 4, c // 4
        in2.append({
            "p0": np.ascontiguousarray(parts[b][eh * DL:(eh + 1) * DL]),
            "p1": np.ascontiguousarray(parts[4 + b][eh * DL:(eh + 1) * DL]),
        })
    res2 = run_bass_kernel_spmd(_cache["s2"], in2, core_ids=list(range(8)), **kw)

    outs = []
    for b in range(4):
        s0 = res2.results[b]["s"].transpose(1, 0, 2).reshape(DL, T)
        s1 = res2.results[4 + b]["s"].transpose(1, 0, 2).reshape(DL, T)
        outs.append(np.concatenate([s0, s1], axis=0).T)
    out = np.stack(outs).astype(np.float32)
    if _trace:
        return out, (res1, res2)
    return out
